# revision 12
# baseline (speedup 1.0000x reference)
"""Trainium2 Bass kernel for nn_Attention_36137854828870.

Multi-head causal attention with rotary embeddings:
  y = softmax((rope(x@wq) @ rope(x@wk)^T)/sqrt(hd) + causal) @ (x@wv) @ wo

Sharding (8 cores): data-parallel over batch (4) x tensor-parallel over
heads (2 groups of 8).  Core c handles batch c//2, head group c%2: it gets
column slices of wq/wk/wv and the matching row slice of wo, produces a
partial (S, D) output, and the host sums the two partials per batch
(cheaper than an in-kernel all-reduce at this size).

Per-core kernel (everything transposed so no on-chip transposes needed):
  1. Stream xT s-chunks; QT/KT = wq/wk-tile.T @ xT (d on partitions),
     V = xT-tile.T @ wv (s on partitions, with a ones column per head for
     the softmax denominator).  Per chunk: RoPE on QT/KT in a
     rope-friendly permutation (even dims of all heads in partition-tiles
     0-1, odd dims in 2-3, so pairs are lane-aligned), then DMA-shuffle
     the chunk to a head-contiguous layout in DRAM (QTb/KTb).
  2. Attention, q-block (1024) outer, head-pair inner, K/Q streamed back
     from DRAM: scoresT = K_h-tile.T @ QT (keys on partitions), exp on
     ScalarE with 1/sqrt(hd) folded into the activation scale (no
     max-subtraction: |scores| is tiny so fp32 exp is exact), causal mask
     on diagonal tiles via gpsimd.affine_select, P@V accumulated in PSUM
     (M=65: 64 head dims + denominator row).  The j-loop is software
     pipelined (PV(j-1) is emitted after scores(j)/exp(j)) so the PE
     never in-order-stalls on the ScalarE exp.  1/l normalization is
     fused into the PSUM eviction (denominator row broadcast across
     partitions with a 0-stride DMA).
  3. y = attnT-tile.T @ wo, DMA out.

All matmuls run as float32r (fp32 bits, reduced-precision multiply at
full PE rate); accumulation is fp32 in PSUM.
"""

import sys

sys.path.insert(0, "/opt/trn_rl_repo")

import numpy as np

import concourse.bass as bass
import concourse.mybir as mybir
import concourse.tile as tile
from concourse import bacc
from concourse.bass_utils import run_bass_kernel_spmd

B, S, D = 4, 2048, 1024
H, HD = 16, 64
P = 128
NCORES = 8
HPC = H // 2          # heads per core
DG = HPC * HD         # 512: per-core head-group width
NKT = D // P          # 8 contraction tiles for projections
NDT = DG // P         # 4 partition-tiles of QT/KT
NSC = S // 512        # 4 s-chunks
NST = S // P          # 16 s(key)-tiles
QW = 1024             # attention q-block width
NQB = S // QW         # 2 q-blocks
F32 = mybir.dt.float32
F32R = mybir.dt.float32r

_PROGRAM = None


def _r(ap):
    return ap.bitcast(F32R)


def _build_program():
    nc = bacc.Bacc("TRN2", target_bir_lowering=False, debug=False)

    xT_d = nc.dram_tensor("xT", [D, S], F32R, kind="ExternalInput")
    wq_d = nc.dram_tensor("wq", [D, DG], F32R, kind="ExternalInput")
    wk_d = nc.dram_tensor("wk", [D, DG], F32R, kind="ExternalInput")
    wv_d = nc.dram_tensor("wv", [D, DG], F32R, kind="ExternalInput")
    wo_d = nc.dram_tensor("wo", [DG, D], F32R, kind="ExternalInput")
    cos_d = nc.dram_tensor("cost", [P, S], F32, kind="ExternalInput")
    sin_d = nc.dram_tensor("sint", [P, S], F32, kind="ExternalInput")
    y_d = nc.dram_tensor("y", [S, D], F32, kind="ExternalOutput")
    # permB (head-contiguous) Q/K staging in DRAM: [dtb, p, s]
    qtb_d = nc.dram_tensor("qtb_i", [NDT, P, S], F32R)
    ktb_d = nc.dram_tensor("ktb_i", [NDT, P, S], F32R)

    xT_v = xT_d.ap().rearrange("(kt p) s -> p kt s", p=P)
    wq_v = wq_d.ap().rearrange("(kt p) m -> p kt m", p=P)
    wk_v = wk_d.ap().rearrange("(kt p) m -> p kt m", p=P)
    wv_v = wv_d.ap().rearrange("(kt p) m -> p kt m", p=P)
    wo_v = wo_d.ap().rearrange("(dt p) n -> p dt n", p=P)

    with tile.TileContext(nc) as tc:
        with tc.tile_pool(name="vpool", bufs=1) as vpool:
            V = vpool.tile([P, NST, HPC, HD + 1], F32R, tag="V")
            ones = vpool.tile([P, NST * HPC], F32, tag="ones")
            nc.any.memset(ones[:], 1.0)
            nc.vector.tensor_copy(
                V[:, :, :, HD : HD + 1],
                ones[:].rearrange("p (a b) -> p a b", a=NST),
            )

            # ---- phase 1: projections + rope + shuffle, per s-chunk ----
            with tc.tile_pool(name="projout", bufs=1) as projout, \
                 tc.tile_pool(name="wres", bufs=1) as wres, \
                 tc.tile_pool(name="xw", bufs=2) as xw, \
                 tc.tile_pool(name="trig", bufs=1) as trig, \
                 tc.tile_pool(name="scr", bufs=1) as scr, \
                 tc.tile_pool(name="ps1", bufs=3, space="PSUM") as ps1:
                QT = projout.tile([P, NDT, S], F32, tag="QT")
                KT = projout.tile([P, NDT, S], F32, tag="KT")
                wqt = wres.tile([P, NKT, DG], F32R, tag="wq")
                wkt = wres.tile([P, NKT, DG], F32R, tag="wk")
                wvt = wres.tile([P, NKT, DG], F32R, tag="wv")
                nc.sync.dma_start(out=wqt[:], in_=wq_v[:])
                nc.sync.dma_start(out=wkt[:], in_=wk_v[:])
                nc.sync.dma_start(out=wvt[:], in_=wv_v[:])
                cost = trig.tile([P, S], F32, tag="cos")
                sint = trig.tile([P, S], F32, tag="sin")
                nc.sync.dma_start(out=cost[:], in_=cos_d.ap())
                nc.sync.dma_start(out=sint[:], in_=sin_d.ap())

                for c in range(NSC):
                    csl = slice(c * 512, (c + 1) * 512)
                    xc = xw.tile([P, NKT, 512], F32R, tag="xc")
                    nc.sync.dma_start(out=xc[:], in_=xT_v[:, :, csl])
                    for wt, out_t in ((wqt, QT), (wkt, KT)):
                        for dt in range(NDT):
                            psq = ps1.tile([P, 512], F32, tag="ps")
                            for kt in range(NKT):
                                nc.tensor.matmul(
                                    psq[:],
                                    wt[:, kt, dt * P : (dt + 1) * P],
                                    xc[:, kt, :],
                                    start=(kt == 0),
                                    stop=(kt == NKT - 1),
                                )
                            nc.scalar.copy(out_t[:, dt, csl], psq[:])
                    for st in range(4):
                        psv = ps1.tile([P, 512], F32, tag="ps")
                        for kt in range(NKT):
                            nc.tensor.matmul(
                                psv[:],
                                xc[:, kt, st * P : (st + 1) * P],
                                wvt[:, kt, :],
                                start=(kt == 0),
                                stop=(kt == NKT - 1),
                            )
                        nc.vector.tensor_copy(
                            V[:, c * 4 + st, :, 0:HD],
                            psv[:].rearrange("p (h d) -> p h d", h=HPC),
                        )

                    # rope this chunk (permA: dt 0-1 = even dims, 2-3 = odd)
                    for t in (QT, KT):
                        for dt in range(2):
                            a0 = t[:, dt, csl]
                            a1 = t[:, dt + 2, csl]
                            cc = cost[:, csl]
                            ss = sint[:, csl]
                            tt = scr.tile([P, 512], F32, tag="t")
                            uu = scr.tile([P, 512], F32, tag="u")
                            nc.vector.tensor_mul(tt[:], a0, ss)
                            nc.vector.tensor_mul(uu[:], a1, cc)
                            nc.vector.tensor_mul(a0, a0, cc)
                            nc.vector.tensor_mul(a1, a1, ss)
                            nc.vector.tensor_sub(a0, a0, a1)
                            nc.vector.tensor_add(a1, tt[:], uu[:])

                    # shuffle chunk permA -> permB (head-contiguous) in DRAM
                    # permA: head h even dims at (dt=h//4, p=(h%4)*32), odd
                    # at dt+2.  permB: head h at (dtb=h//2, p=(h%2)*64).
                    for src, dst in ((QT, qtb_d), (KT, ktb_d)):
                        for h in range(HPC):
                            pa = (h % 4) * 32
                            pb = (h % 2) * 64
                            for half in range(2):
                                nc.sync.dma_start(
                                    out=dst.ap()[
                                        h // 2,
                                        pb + 32 * half : pb + 32 * half + 32,
                                        csl,
                                    ],
                                    in_=_r(src[pa : pa + 32, half * 2 + h // 4, csl]),
                                )

            # ---- phase 2: attention ----
            with tc.tile_pool(name="atpool", bufs=1) as atpool:
              attnT = atpool.tile([P, NDT, S], F32R, tag="attnT")
              with tc.tile_pool(name="qkst", bufs=2) as qkst, \
                   tc.tile_pool(name="apsum", bufs=2, space="PSUM") as apsum, \
                   tc.tile_pool(name="opsum", bufs=2, space="PSUM") as opsum, \
                   tc.tile_pool(name="expool", bufs=4) as expool, \
                   tc.tile_pool(name="npool", bufs=2) as npool:
                for qb in range(NQB):
                    kr = (qb + 1) * QW      # causal key range for this block
                    njt = kr // P
                    for dtb in range(NDT):  # head pair (2*dtb, 2*dtb+1)
                        kst = qkst.tile([P, S], F32R, tag="kst")
                        nc.sync.dma_start(out=kst[:, 0:kr], in_=ktb_d.ap()[dtb, :, 0:kr])
                        qst = qkst.tile([P, QW], F32R, tag="qst")
                        nc.sync.dma_start(
                            out=qst[:], in_=qtb_d.ap()[dtb, :, qb * QW : (qb + 1) * QW]
                        )
                        for hh in range(2):
                            pb = hh * 64
                            h = dtb * 2 + hh
                            pso = opsum.tile([P, QW], F32, tag="pso")

                            def emit_pv(j, pieces, ex):
                                for lo, hi in pieces:
                                    nc.tensor.matmul(
                                        pso[0 : HD + 1, lo:hi],
                                        V[:, j, h, :],
                                        ex[:, lo:hi],
                                        start=(j == 0),
                                        stop=(j == njt - 1),
                                    )

                            prev = None
                            for j in range(njt):
                                diag = j >= njt - (QW // P)
                                qlo = (j - (njt - QW // P)) * P if diag else 0
                                pieces = (
                                    [(qlo, 512), (512, QW)]
                                    if qlo < 512
                                    else [(qlo, QW)]
                                )
                                pss = apsum.tile([P, QW], F32, tag="pss")
                                for lo, hi in pieces:
                                    nc.tensor.matmul(
                                        pss[:, lo:hi],
                                        kst[pb : pb + 64, j * P : (j + 1) * P],
                                        qst[pb : pb + 64, lo:hi],
                                        start=True,
                                        stop=True,
                                    )
                                ex = expool.tile([P, QW], F32R, tag="ex")
                                nc.scalar.activation(
                                    ex[:, qlo:QW],
                                    pss[:, qlo:QW],
                                    mybir.ActivationFunctionType.Exp,
                                    scale=float(1.0 / np.sqrt(HD)),
                                )
                                if diag:
                                    nc.gpsimd.affine_select(
                                        out=ex[:, qlo : qlo + P],
                                        in_=ex[:, qlo : qlo + P],
                                        compare_op=mybir.AluOpType.is_ge,
                                        fill=0.0,
                                        base=0,
                                        pattern=[[1, P]],
                                        channel_multiplier=-1,
                                    )
                                if prev is not None:
                                    emit_pv(*prev)
                                prev = (j, pieces, ex)
                            emit_pv(*prev)

                            lr = npool.tile([P, QW], F32, tag="lr")
                            nc.vector.reciprocal(lr[HD : HD + 1, :], pso[HD : HD + 1, :])
                            bc = npool.tile([P, QW], F32, tag="bc")
                            nc.sync.dma_start(
                                out=bc[0:HD, :],
                                in_=lr[HD : HD + 1, :]
                                .unsqueeze(1)
                                .broadcast_to((1, HD, QW)),
                            )
                            nc.vector.tensor_mul(
                                attnT[pb : pb + HD, dtb, qb * QW : (qb + 1) * QW],
                                pso[0:HD, :],
                                bc[0:HD, :],
                            )

              # ---- phase 3: output projection ----
              with tc.tile_pool(name="wop", bufs=1) as wop, \
                   tc.tile_pool(name="ypool", bufs=3) as ypool, \
                   tc.tile_pool(name="ps5", bufs=3, space="PSUM") as ps5:
                    wo_sb = wop.tile([P, NDT, D], F32R, tag="wo")
                    nc.sync.dma_start(out=wo_sb[:], in_=wo_v[:])
                    for qt16 in range(NST):
                        for nt in range(2):
                            psy = ps5.tile([P, 512], F32, tag="ps")
                            for dt in range(NDT):
                                nc.tensor.matmul(
                                    psy[:],
                                    attnT[:, dt, qt16 * P : (qt16 + 1) * P],
                                    wo_sb[:, dt, nt * 512 : (nt + 1) * 512],
                                    start=(dt == 0),
                                    stop=(dt == NDT - 1),
                                )
                            yt = ypool.tile([P, 512], F32, tag="yt")
                            nc.any.tensor_copy(yt[:], psy[:])
                            nc.sync.dma_start(
                                out=y_d.ap()[
                                    qt16 * P : (qt16 + 1) * P, nt * 512 : (nt + 1) * 512
                                ],
                                in_=yt[:],
                            )

    nc.compile()
    return nc


def _perm_a():
    """Column permutation for wq/wk: even head-dims of all heads first
    (head-major, 32 per head), then odd head-dims."""
    perm = np.empty(DG, dtype=np.int64)
    for n in range(DG):
        if n < DG // 2:
            h, i = n // 32, n % 32
            perm[n] = h * HD + 2 * i
        else:
            h, i = (n - DG // 2) // 32, (n - DG // 2) % 32
            perm[n] = h * HD + 2 * i + 1
    return perm


def kernel(**inputs):
    global _PROGRAM
    x = np.asarray(inputs["x"], dtype=np.float32)
    freqs_cos = np.asarray(inputs["freqs_cos"], dtype=np.float32)
    freqs_sin = np.asarray(inputs["freqs_sin"], dtype=np.float32)
    wq = np.asarray(inputs["wq"], dtype=np.float32)
    wk = np.asarray(inputs["wk"], dtype=np.float32)
    wv = np.asarray(inputs["wv"], dtype=np.float32)
    wo = np.asarray(inputs["wo"], dtype=np.float32)

    if _PROGRAM is None:
        _PROGRAM = _build_program()
    nc = _PROGRAM

    perm = _perm_a()
    # cos/sin tables: (S, HD//2) -> (128, S), row p holds cos[:, p % 32]
    cost = np.ascontiguousarray(np.tile(freqs_cos.T, (4, 1)))
    sint = np.ascontiguousarray(np.tile(freqs_sin.T, (4, 1)))

    in_maps = []
    for c in range(NCORES):
        b, g = c // 2, c % 2
        gsl = slice(g * DG, (g + 1) * DG)
        in_maps.append(
            {
                "xT": np.ascontiguousarray(x[b].T),
                "wq": np.ascontiguousarray(wq[:, gsl][:, perm]),
                "wk": np.ascontiguousarray(wk[:, gsl][:, perm]),
                "wv": np.ascontiguousarray(wv[:, gsl]),
                "wo": np.ascontiguousarray(wo[gsl, :]),
                "cost": cost,
                "sint": sint,
            }
        )

    res = run_bass_kernel_spmd(nc, in_maps, list(range(NCORES)))
    y = np.empty((B, S, D), dtype=np.float32)
    for b in range(B):
        y[b] = res.results[2 * b]["y"] + res.results[2 * b + 1]["y"]
    return y


# revision 15
# speedup vs baseline: 1.2079x; 1.2079x over previous
"""Trainium2 Bass kernel for nn_Attention_36137854828870.

Multi-head causal attention with rotary embeddings:
  y = softmax((rope(x@wq) @ rope(x@wk)^T)/sqrt(hd) + causal) @ (x@wv) @ wo

Sharding (8 cores): data-parallel over batch (4) x tensor-parallel over
heads (2 groups of 8).  Core c handles batch c//2, head group c%2: it gets
column slices of wq/wk/wv and the matching row slice of wo, produces a
partial (S, D) output, and the host sums the two partials per batch
(cheaper than an in-kernel all-reduce at this size).

Per-core kernel (everything transposed so no on-chip transposes needed):
  1. Stream xT s-chunks; QT/KT = wq/wk-tile.T @ xT (d on partitions),
     V = xT-tile.T @ wv (s on partitions, with a ones column per head for
     the softmax denominator).  Per chunk: RoPE on QT/KT in a
     rope-friendly permutation (even dims of all heads in partition-tiles
     0-1, odd dims in 2-3, so pairs are lane-aligned), then DMA-shuffle
     the chunk to a head-contiguous layout in DRAM (QTb/KTb).
  2. Attention, q-block (1024) outer, head-pair inner, K/Q streamed back
     from DRAM: scoresT = K_h-tile.T @ QT (keys on partitions), exp on
     ScalarE with 1/sqrt(hd) folded into the activation scale (no
     max-subtraction: |scores| is tiny so fp32 exp is exact), causal mask
     on diagonal tiles via gpsimd.affine_select, P@V accumulated in PSUM
     (M=65: 64 head dims + denominator row).  The j-loop is software
     pipelined two deep (PV(j-2) is emitted after scores(j)/exp(j)) so
     neither PE nor ScalarE in-order-stalls on the other.  PSUM is
     evicted unnormalized (reciprocal of the denominator row lands in a
     small l-tile); the 1/l scale is applied afterwards, off the critical
     path, with a 0-stride broadcast DMA + in-place multiply.
  3. y = attnT-tile.T @ wo, DMA out.

All matmuls run as float32r (fp32 bits, reduced-precision multiply at
full PE rate); accumulation is fp32 in PSUM.  DMAs are spread across the
sync (loads) and gpsimd (shuffle/broadcast/store) queues so dispatch
doesn't serialize behind one sequencer.
"""

import sys

sys.path.insert(0, "/opt/trn_rl_repo")

import numpy as np

import concourse.bass as bass
import concourse.mybir as mybir
import concourse.tile as tile
from concourse import bacc
from concourse.bass_utils import run_bass_kernel_spmd

B, S, D = 4, 2048, 1024
H, HD = 16, 64
P = 128
NCORES = 8
HPC = H // 2          # heads per core
DG = HPC * HD         # 512: per-core head-group width
NKT = D // P          # 8 contraction tiles for projections
NDT = DG // P         # 4 partition-tiles of QT/KT
NSC = S // 512        # 4 s-chunks
NST = S // P          # 16 s(key)-tiles
QW = 1024             # attention q-block width
NQB = S // QW         # 2 q-blocks
F32 = mybir.dt.float32
F32R = mybir.dt.float32r

_PROGRAM = None


def _r(ap):
    return ap.bitcast(F32R)


def _build_program():
    nc = bacc.Bacc("TRN2", target_bir_lowering=False, debug=False)

    xT_d = nc.dram_tensor("xT", [D, S], F32R, kind="ExternalInput")
    wq_d = nc.dram_tensor("wq", [D, DG], F32R, kind="ExternalInput")
    wk_d = nc.dram_tensor("wk", [D, DG], F32R, kind="ExternalInput")
    wv_d = nc.dram_tensor("wv", [D, DG], F32R, kind="ExternalInput")
    wo_d = nc.dram_tensor("wo", [DG, D], F32R, kind="ExternalInput")
    cos_d = nc.dram_tensor("cost", [P, S], F32, kind="ExternalInput")
    sin_d = nc.dram_tensor("sint", [P, S], F32, kind="ExternalInput")
    y_d = nc.dram_tensor("y", [S, D], F32, kind="ExternalOutput")
    # permB (head-contiguous) Q/K staging in DRAM, viewed as 8 half-tiles
    # of 64 partitions: half-tile h holds head h's 64 dims.
    qtb_d = nc.dram_tensor("qtb_i", [NDT, P, S], F32R)
    ktb_d = nc.dram_tensor("ktb_i", [NDT, P, S], F32R)

    xT_v = xT_d.ap().rearrange("(kt p) s -> p kt s", p=P)
    wq_v = wq_d.ap().rearrange("(kt p) m -> p kt m", p=P)
    wk_v = wk_d.ap().rearrange("(kt p) m -> p kt m", p=P)
    wv_v = wv_d.ap().rearrange("(kt p) m -> p kt m", p=P)
    wo_v = wo_d.ap().rearrange("(dt p) n -> p dt n", p=P)
    # (8 half-tiles, 64, S) views for the shuffle destinations
    qtb8 = qtb_d.ap().rearrange("dtb (ht p) s -> (dtb ht) p s", ht=2)
    ktb8 = ktb_d.ap().rearrange("dtb (ht p) s -> (dtb ht) p s", ht=2)

    with tile.TileContext(nc) as tc:
        with tc.tile_pool(name="vpool", bufs=1) as vpool:
            V = vpool.tile([P, NST, HPC, HD + 1], F32R, tag="V")
            ones = vpool.tile([P, NST * HPC], F32, tag="ones")
            nc.any.memset(ones[:], 1.0)
            nc.vector.tensor_copy(
                V[:, :, :, HD : HD + 1],
                ones[:].rearrange("p (a b) -> p a b", a=NST),
            )

            # ---- phase 1: projections + rope + shuffle, per s-chunk ----
            with tc.tile_pool(name="projout", bufs=1) as projout, \
                 tc.tile_pool(name="wres", bufs=1) as wres, \
                 tc.tile_pool(name="xw", bufs=2) as xw, \
                 tc.tile_pool(name="trig", bufs=1) as trig, \
                 tc.tile_pool(name="scr", bufs=1) as scr, \
                 tc.tile_pool(name="ps1", bufs=3, space="PSUM") as ps1:
                QT = projout.tile([P, NDT, S], F32, tag="QT")
                KT = projout.tile([P, NDT, S], F32, tag="KT")
                wqt = wres.tile([P, NKT, DG], F32R, tag="wq")
                wkt = wres.tile([P, NKT, DG], F32R, tag="wk")
                wvt = wres.tile([P, NKT, DG], F32R, tag="wv")
                cost = trig.tile([P, S], F32, tag="cos")
                sint = trig.tile([P, S], F32, tag="sin")
                # split loads, in consumption order, so the first matmuls
                # only wait on xc + wq
                xc0 = xw.tile([P, NKT, 512], F32R, tag="xc")
                nc.sync.dma_start(out=xc0[:], in_=xT_v[:, :, 0:512])
                for dt in range(NDT):
                    nc.sync.dma_start(
                        out=wqt[:, :, dt * P : (dt + 1) * P],
                        in_=wq_v[:, :, dt * P : (dt + 1) * P],
                    )
                for dt in range(NDT):
                    nc.sync.dma_start(
                        out=wkt[:, :, dt * P : (dt + 1) * P],
                        in_=wk_v[:, :, dt * P : (dt + 1) * P],
                    )
                nc.sync.dma_start(out=wvt[:], in_=wv_v[:])
                nc.gpsimd.dma_start(out=cost[:], in_=cos_d.ap())
                nc.gpsimd.dma_start(out=sint[:], in_=sin_d.ap())

                for c in range(NSC):
                    csl = slice(c * 512, (c + 1) * 512)
                    if c == 0:
                        xc = xc0
                    else:
                        xc = xw.tile([P, NKT, 512], F32R, tag="xc")
                        nc.sync.dma_start(out=xc[:], in_=xT_v[:, :, csl])
                    for wt, out_t in ((wqt, QT), (wkt, KT)):
                        for dt in range(NDT):
                            psq = ps1.tile([P, 512], F32, tag="ps")
                            for kt in range(NKT):
                                nc.tensor.matmul(
                                    psq[:],
                                    wt[:, kt, dt * P : (dt + 1) * P],
                                    xc[:, kt, :],
                                    start=(kt == 0),
                                    stop=(kt == NKT - 1),
                                )
                            nc.scalar.copy(out_t[:, dt, csl], psq[:])
                    for st in range(4):
                        psv = ps1.tile([P, 512], F32, tag="ps")
                        for kt in range(NKT):
                            nc.tensor.matmul(
                                psv[:],
                                xc[:, kt, st * P : (st + 1) * P],
                                wvt[:, kt, :],
                                start=(kt == 0),
                                stop=(kt == NKT - 1),
                            )
                        nc.vector.tensor_copy(
                            V[:, c * 4 + st, :, 0:HD],
                            psv[:].rearrange("p (h d) -> p h d", h=HPC),
                        )

                    # rope this chunk (permA: dt 0-1 = even dims, 2-3 = odd)
                    for t in (QT, KT):
                        for dt in range(2):
                            a0 = t[:, dt, csl]
                            a1 = t[:, dt + 2, csl]
                            cc = cost[:, csl]
                            ss = sint[:, csl]
                            tt = scr.tile([P, 512], F32, tag="t")
                            uu = scr.tile([P, 512], F32, tag="u")
                            nc.vector.tensor_mul(tt[:], a0, ss)
                            nc.vector.tensor_mul(uu[:], a1, cc)
                            nc.vector.tensor_mul(a0, a0, cc)
                            nc.vector.tensor_mul(a1, a1, ss)
                            nc.vector.tensor_sub(a0, a0, a1)
                            nc.vector.tensor_add(a1, tt[:], uu[:])

                    # shuffle chunk permA -> permB (head-contiguous) in DRAM.
                    # permA partition-tile dt holds 4 heads' 32-row strips;
                    # strip (h%4) of tile dt maps to half-tile h, rows
                    # [0,32) for evens (dt<2) or [32,64) for odds.
                    for src, dst8 in ((QT, qtb8), (KT, ktb8)):
                        for dt in range(NDT):
                            hbase = 4 * (dt % 2)
                            rlo = 32 * (dt // 2)
                            nc.gpsimd.dma_start(
                                out=dst8[hbase : hbase + 4, rlo : rlo + 32, csl],
                                in_=_r(src[:, dt, csl]),
                            )

            # ---- phase 2: attention ----
            with tc.tile_pool(name="atpool", bufs=1) as atpool:
                attnT = atpool.tile([P, NDT, S], F32R, tag="attnT")
                ltile = atpool.tile([P, 2, S], F32, tag="ltile")
                with tc.tile_pool(name="qkst", bufs=2) as qkst, \
                     tc.tile_pool(name="apsum", bufs=3, space="PSUM") as apsum, \
                     tc.tile_pool(name="opsum", bufs=1, space="PSUM") as opsum, \
                     tc.tile_pool(name="expool", bufs=6) as expool, \
                     tc.tile_pool(name="npool", bufs=2) as npool:
                    for qb in range(NQB):
                        kr = (qb + 1) * QW
                        njt = kr // P
                        qsl = slice(qb * QW, (qb + 1) * QW)
                        for dtb in range(NDT):  # head pair (2*dtb, 2*dtb+1)
                            kst = qkst.tile([P, S], F32R, tag="kst")
                            nc.sync.dma_start(
                                out=kst[:, 0:kr], in_=ktb_d.ap()[dtb, :, 0:kr]
                            )
                            qst = qkst.tile([P, QW], F32R, tag="qst")
                            nc.sync.dma_start(out=qst[:], in_=qtb_d.ap()[dtb, :, qsl])
                            for hh in range(2):
                                pb = hh * 64
                                h = dtb * 2 + hh
                                pso = opsum.tile([P, QW], F32, tag="pso")

                                def emit_pv(j, pieces, ex):
                                    for lo, hi in pieces:
                                        nc.tensor.matmul(
                                            pso[0 : HD + 1, lo:hi],
                                            V[:, j, h, :],
                                            ex[:, lo:hi],
                                            start=(j == 0),
                                            stop=(j == njt - 1),
                                        )

                                pipe = []
                                for j in range(njt):
                                    diag = j >= njt - (QW // P)
                                    qlo = (j - (njt - QW // P)) * P if diag else 0
                                    pieces = (
                                        [(qlo, 512), (512, QW)]
                                        if qlo < 512
                                        else [(qlo, QW)]
                                    )
                                    pss = apsum.tile([P, QW], F32, tag="pss")
                                    for lo, hi in pieces:
                                        nc.tensor.matmul(
                                            pss[:, lo:hi],
                                            kst[pb : pb + 64, j * P : (j + 1) * P],
                                            qst[pb : pb + 64, lo:hi],
                                            start=True,
                                            stop=True,
                                        )
                                    ex = expool.tile([P, QW], F32R, tag="ex")
                                    nc.scalar.activation(
                                        ex[:, qlo:QW],
                                        pss[:, qlo:QW],
                                        mybir.ActivationFunctionType.Exp,
                                        scale=float(1.0 / np.sqrt(HD)),
                                    )
                                    if diag:
                                        nc.gpsimd.affine_select(
                                            out=ex[:, qlo : qlo + P],
                                            in_=ex[:, qlo : qlo + P],
                                            compare_op=mybir.AluOpType.is_ge,
                                            fill=0.0,
                                            base=0,
                                            pattern=[[1, P]],
                                            channel_multiplier=-1,
                                        )
                                    pipe.append((j, pieces, ex))
                                    if len(pipe) > 2:
                                        emit_pv(*pipe.pop(0))
                                for item in pipe:
                                    emit_pv(*item)

                                # unnormalized eviction; 1/l into the l-tile
                                nc.vector.reciprocal(
                                    ltile[(h % 4) * 32 : (h % 4) * 32 + 1, h // 4, qsl],
                                    pso[HD : HD + 1, :],
                                )
                                nc.vector.tensor_copy(
                                    attnT[pb : pb + HD, dtb, qsl], pso[0:HD, :]
                                )

                    # deferred normalization: attnT *= broadcast(1/l)
                    for qb in range(NQB):
                        qsl = slice(qb * QW, (qb + 1) * QW)
                        for h in range(HPC):
                            pb = (h % 2) * 64
                            dtb = h // 2
                            bc = npool.tile([P, QW], F32, tag="bc")
                            nc.gpsimd.dma_start(
                                out=bc[pb : pb + HD, :],
                                in_=ltile[
                                    (h % 4) * 32 : (h % 4) * 32 + 1, h // 4, qsl
                                ]
                                .unsqueeze(1)
                                .broadcast_to((1, HD, QW)),
                            )
                            nc.vector.tensor_mul(
                                attnT[pb : pb + HD, dtb, qsl],
                                attnT[pb : pb + HD, dtb, qsl],
                                bc[pb : pb + HD, :],
                            )

                # ---- phase 3: output projection ----
                with tc.tile_pool(name="wop", bufs=1) as wop, \
                     tc.tile_pool(name="ypool", bufs=3) as ypool, \
                     tc.tile_pool(name="ps5", bufs=3, space="PSUM") as ps5:
                    wo_sb = wop.tile([P, NDT, D], F32R, tag="wo")
                    nc.sync.dma_start(out=wo_sb[:], in_=wo_v[:])
                    for qt16 in range(NST):
                        for nt in range(2):
                            psy = ps5.tile([P, 512], F32, tag="ps")
                            for dt in range(NDT):
                                nc.tensor.matmul(
                                    psy[:],
                                    attnT[:, dt, qt16 * P : (qt16 + 1) * P],
                                    wo_sb[:, dt, nt * 512 : (nt + 1) * 512],
                                    start=(dt == 0),
                                    stop=(dt == NDT - 1),
                                )
                            yt = ypool.tile([P, 512], F32, tag="yt")
                            nc.scalar.copy(yt[:], psy[:])
                            nc.gpsimd.dma_start(
                                out=y_d.ap()[
                                    qt16 * P : (qt16 + 1) * P, nt * 512 : (nt + 1) * 512
                                ],
                                in_=yt[:],
                            )

    nc.compile()
    return nc


def _perm_a():
    """Column permutation for wq/wk: even head-dims of all heads first
    (head-major, 32 per head), then odd head-dims."""
    perm = np.empty(DG, dtype=np.int64)
    for n in range(DG):
        if n < DG // 2:
            h, i = n // 32, n % 32
            perm[n] = h * HD + 2 * i
        else:
            h, i = (n - DG // 2) // 32, (n - DG // 2) % 32
            perm[n] = h * HD + 2 * i + 1
    return perm


def kernel(**inputs):
    global _PROGRAM
    x = np.asarray(inputs["x"], dtype=np.float32)
    freqs_cos = np.asarray(inputs["freqs_cos"], dtype=np.float32)
    freqs_sin = np.asarray(inputs["freqs_sin"], dtype=np.float32)
    wq = np.asarray(inputs["wq"], dtype=np.float32)
    wk = np.asarray(inputs["wk"], dtype=np.float32)
    wv = np.asarray(inputs["wv"], dtype=np.float32)
    wo = np.asarray(inputs["wo"], dtype=np.float32)

    if _PROGRAM is None:
        _PROGRAM = _build_program()
    nc = _PROGRAM

    perm = _perm_a()
    # cos/sin tables: (S, HD//2) -> (128, S), row p holds cos[:, p % 32]
    cost = np.ascontiguousarray(np.tile(freqs_cos.T, (4, 1)))
    sint = np.ascontiguousarray(np.tile(freqs_sin.T, (4, 1)))

    in_maps = []
    for c in range(NCORES):
        b, g = c // 2, c % 2
        gsl = slice(g * DG, (g + 1) * DG)
        in_maps.append(
            {
                "xT": np.ascontiguousarray(x[b].T),
                "wq": np.ascontiguousarray(wq[:, gsl][:, perm]),
                "wk": np.ascontiguousarray(wk[:, gsl][:, perm]),
                "wv": np.ascontiguousarray(wv[:, gsl]),
                "wo": np.ascontiguousarray(wo[gsl, :]),
                "cost": cost,
                "sint": sint,
            }
        )

    res = run_bass_kernel_spmd(nc, in_maps, list(range(NCORES)))
    y = np.empty((B, S, D), dtype=np.float32)
    for b in range(B):
        y[b] = res.results[2 * b]["y"] + res.results[2 * b + 1]["y"]
    return y
